# revision 43
# baseline (speedup 1.0000x reference)
"""Trainium2 Bass kernel for the SE-sweep DAG-RNN (nn_DAG_RNN_se).

Reference semantics (B=32, C=512, H=W=32):
    h[i,j] = relu(x[:,:,i,j] + (h[i-1,j] + h[i,j-1]) @ W_hh)     # [B, C]
    y[i,j] = h[i,j] @ W_yh + bias

Strategy:
  * Data-parallel over batch: 8 cores x 4 batch elements, zero communication.
  * Anti-diagonal wavefront inside a core: diagonal d holds n_d cells; all
    cells of a diagonal are batched into one set of matmuls.
  * State layout is transposed: h^T [C(4x128 partitions), n_d*B_local] so
    W_hh chunks are the stationary matmul operand; N = 4*n_d <= 128.
  * The kernel is LDWEIGHTS-issue bound (~73ns per matmul regardless of
    N<=128), so x-injection identity matmuls are eliminated: a custom DVE
    op RELU_ADD_SE computes h = relu(psum + x) in one Vector instruction
    per chunk pair.
  * PSUM pair tiles span two banks (one bank per chunk): two interleaved
    accumulation groups must never share a bank (same-bank interleaved
    groups corrupt PSUM).
  * The hidden-state buffer hj is laid out with a BL-wide ZERO gap between
    diagonals (memset once at startup): each next-diagonal hsum chunk is
    then ONE plain shifted tensor-tensor add whose first/last cells pick
    up the zero pad — no boundary ops, and the 2D fp16 SBUF op gets DVE
    2x mode.  DVE order [customA, stt0, stt1, customB, stt2, stt3] lets
    the next diagonal's k=0 matmuls start ~2 ops after the pair-A psum
    closes.
  * y = h @ W_yh work (512-wide matmuls whose LDWEIGHTS hide under the
    moving stream) is emitted inline behind the wavefront in 4-matmul
    ct-groups — small enough to never head-of-line-block the recurrence.
  * y is stored and DMA'd as fp16 (half the write traffic); the host
    upcasts. fp16 state + weights; PSUM accumulates fp32.

The full (unsharded) numpy contract is `kernel(**inputs)`; the Bass program
is built and compiled once and cached at module level.
"""

import sys

if "/opt/trn_rl_repo" not in sys.path:
    sys.path.insert(0, "/opt/trn_rl_repo")

import numpy as np

import concourse.bass as bass
import concourse.mybir as mybir
import concourse.tile as tile
from concourse import bacc
from concourse import bass_utils

# ---------------------------------------------------------------- constants
B, C, H, W = 32, 512, 32, 32
NCORES = 8
BL = B // NCORES            # local batch per core = 4
ND = H + W - 1              # 63 diagonals
CT = 4                      # channel chunks of 128
P = 128
SLOT = 512                  # per-chunk column slot inside a psum pair tile

F32 = mybir.dt.float32
F16 = mybir.dt.float16
ALU = mybir.AluOpType
ACTF = mybir.ActivationFunctionType

N_D = [min(d, H - 1) - max(0, d - (W - 1)) + 1 for d in range(ND)]
IMIN = [max(0, d - (W - 1)) for d in range(ND)]
OFFB = [0] * (ND + 1)
for _d in range(ND):
    OFFB[_d + 1] = OFFB[_d] + N_D[_d] * BL
TOT = OFFB[ND]              # 4096 packed columns (x layout)
# padded h/y layout: BL zero columns before every diagonal
OFF2 = [OFFB[d] + (d + 1) * BL for d in range(ND)]
TOT2 = TOT + (ND + 1) * BL  # 4352

# y output chunks (col0, width) over the padded layout. Narrow 128-wide
# chunks at the start (ready early — they fill the chain-bound expanding
# triangle) and at the very end (so the forced tail after the last
# diagonal is short); 512-wide in the middle (LDWEIGHTS fully hidden).
YCHUNKS = ([(i * 128, 128) for i in range(8)]
           + [(1024 + i * 512, 512) for i in range(5)]
           + [(3584, 384)]
           + [(3968 + i * 128, 128) for i in range(3)])
assert sum(w for _, w in YCHUNKS) == TOT2


def _register_relu_add():
    """Register the fused h = relu(psum + x) DVE op (idempotent)."""
    from concourse import dve_ops
    from concourse.dve_spec import Spec, Src0, Src1, relu, lower, _has_src1
    from concourse.dve_uop import DveOpSpec

    name = "RELU_ADD_SE"
    for op in dve_ops.OPS:
        if op.name == name:
            return op
    spec = Spec(
        body=relu(Src0 + Src1),
        reference=lambda in0, in1, s0, s1, imm2: np.maximum(
            in0.astype(np.float32) + in1.astype(np.float32), 0
        ),
    )
    row = max(dve_ops._SUB_OPCODE_FOR_NAME.values()) + 1
    shas = {}
    for ver in ("v3", "v4"):
        uops = lower(spec, ver=ver)
        shas[ver] = DveOpSpec(
            name=name, opcode=row, uops=uops, rd1_en=_has_src1(spec)
        ).sha(ver)
    op = dve_ops.DveOp(name, spec, subdim=False, uops_sha=shas)
    dve_ops._SUB_OPCODE_FOR_NAME[name] = row
    dve_ops.OPS.append(op)
    return op


def _build_program():
    relu_add = _register_relu_add()

    nc = bacc.Bacc("TRN2", target_bir_lowering=False, debug=False,
                   num_devices=NCORES)

    xs = nc.dram_tensor("xs", [P, CT * TOT], F16, kind="ExternalInput").ap()
    whh = nc.dram_tensor("whh", [C, C], F16, kind="ExternalInput").ap()
    wyh = nc.dram_tensor("wyh", [C, C], F16, kind="ExternalInput").ap()
    biasp = nc.dram_tensor("biasp", [P, CT], F32, kind="ExternalInput").ap()
    y = nc.dram_tensor("y", [C, TOT2], F16, kind="ExternalOutput").ap()

    with tile.TileContext(nc) as tc:
        with (
            tc.tile_pool(name="persist", bufs=1) as persist,
            tc.tile_pool(name="hspool", bufs=3) as hspool,
            tc.tile_pool(name="ypool", bufs=6) as ypool,
            tc.tile_pool(name="recps", bufs=3, space="PSUM") as recps,
            tc.tile_pool(name="yps", bufs=2, space="PSUM") as yps,
        ):
            # ---- resident tensors ----
            whh_sb = persist.tile([P, CT * C], F16, name="whh_sb")
            wyh_sb = persist.tile([P, CT * C], F16, name="wyh_sb")
            bias_sb = persist.tile([P, CT], F32, name="bias_sb")
            # hidden state, chunk-major with BL zero gaps between diagonals
            hj = persist.tile([P, CT * TOT2], F16, name="hj")
            # full input, resident (packed layout, no gaps)
            xsb = persist.tile([P, CT * TOT], F16, name="xsb")

            # Zero the pad columns. Only the expanding-phase STTs ever READ
            # pads (cols < OFF2[31] = 2112); later gap columns are only fed
            # into y matmuls whose outputs at gap positions the host
            # discards, so they may stay uninitialised. A tiny prefix is
            # zeroed up front (unblocks diag 0); the rest is injected into
            # the first diagonals' emission below.

            MZERO = 2112
            MPFX = 128
            for k in range(CT):
                eng = nc.vector if k % 2 == 0 else nc.gpsimd
                eng.memset(hj[:, k * TOT2: k * TOT2 + MPFX], 0.0)

            # Startup: diag 1 needs only a tiny x prefix and W_hh.
            nc.sync.dma_start(xsb[:, 0:64], xs[:, 0:64])
            nc.sync.dma_start(
                whh_sb[:, 0:CT * C].rearrange("p (k c) -> p k c", k=CT),
                whh.rearrange("(k p) c -> p k c", k=CT))
            nc.sync.dma_start(xsb[:, 64:512], xs[:, 64:512])
            nc.sync.dma_start(xsb[:, 512:2048], xs[:, 512:2048])
            NXD = 6
            w = (CT * TOT - 2048) // NXD
            for j in range(NXD):
                c0 = 2048 + j * w
                c1 = CT * TOT if j == NXD - 1 else c0 + w
                eng = nc.sync if j % 2 == 0 else nc.gpsimd
                eng.dma_start(xsb[:, c0:c1], xs[:, c0:c1])
            nc.gpsimd.dma_start(
                wyh_sb[:, 0:CT * C].rearrange("p (k c) -> p k c", k=CT),
                wyh.rearrange("(k p) c -> p k c", k=CT))
            nc.gpsimd.dma_start(bias_sb[:], biasp[:])

            # PE warm-up: the HAM clock gate only reaches 2.4 GHz after
            # ~3.4us of sustained matmul activity, and the startup DMA wait
            # leaves the PE idle for ~6us. Stream dummy matmuls over a
            # zeroed scratch region (high cols of hj chunk 0 — only
            # overwritten by diagonals >= 32, so the WAR is harmless).
            nc.gpsimd.memset(hj[:, MZERO:MZERO + 640], 0.0)
            ps_warm = recps.tile([P, 2 * SLOT], F32, tag="ps",
                                 name="ps_warm")
            for _wi in range(14):
                nc.tensor.matmul(ps_warm[:, 0:512],
                                 lhsT=hj[:, MZERO:MZERO + 128],
                                 rhs=hj[:, MZERO + 128:MZERO + 640],
                                 start=True, stop=True)

            def w_slice(wsb, k, ct):
                return wsb[:, k * C + ct * P: k * C + ct * P + P]

            def hjs(k, c0, wd):
                """h chunk-k cols [c0, c0+wd) as an AP (padded layout)."""
                return hj[:, k * TOT2 + c0: k * TOT2 + c0 + wd]

            def hj2(kbase, c0, wd):
                """strided pair view: chunks kbase,kbase+1, cols [c0,c0+wd)."""
                pair = hj[:, kbase * TOT2:(kbase + 2) * TOT2]
                return pair.rearrange("p (k q) -> p k q", k=2)[:, :, c0:c0 + wd]

            # y bookkeeping: ct-groups emitted inline behind the wavefront
            h_end = OFF2[ND - 1] + N_D[ND - 1] * BL   # last written h col
            YREADY = [min(dd for dd in range(ND) if OFF2[dd] + N_D[dd] * BL
                          >= min(c0 + wd, h_end))
                      for (c0, wd) in YCHUNKS]
            YLAG = 1
            y_queue = [(ci_, ct) for ci_ in range(len(YCHUNKS))
                       for ct in range(CT)]
            y_pos = 0

            def emit_y_group(ci_, ct, pool=None, tail=False):
                c0, wd = YCHUNKS[ci_]
                if pool is None:
                    psy = yps.tile([P, 512], F32, tag="psy",
                                   name=f"psy{c0}_{ct}")
                else:
                    # tail-only: borrow a (free) recurrence psum slot so
                    # more y groups can be in flight while Act does bias
                    psy = recps.tile([P, 2 * SLOT], F32, tag="ps",
                                     name=f"psyr{c0}_{ct}")
                for k in range(CT):
                    nc.tensor.matmul(
                        psy[:, 0:wd],
                        lhsT=w_slice(wyh_sb, k, ct),
                        rhs=hjs(k, c0, wd),
                        start=(k == 0), stop=(k == CT - 1))
                ysb = ypool.tile([P, 512], F16, tag="ysb",
                                 name=f"ysb{c0}_{ct}")
                if tail and wd >= 256:
                    # after the last diagonal the DVE has no chain work:
                    # split the bias add across DVE+Act so the psy slot
                    # recycles twice as fast during the y drain
                    h2 = wd // 2
                    nc.vector.tensor_scalar_add(
                        ysb[:, 0:h2], psy[:, 0:h2], bias_sb[:, ct:ct + 1])
                    nc.scalar.activation(ysb[:, h2:wd], psy[:, h2:wd],
                                         ACTF.Identity,
                                         bias=bias_sb[:, ct:ct + 1],
                                         scale=1.0)
                else:
                    nc.scalar.activation(ysb[:, 0:wd], psy[:, 0:wd],
                                         ACTF.Identity,
                                         bias=bias_sb[:, ct:ct + 1],
                                         scale=1.0)
                # alternate the DMA trigger engine: Pool is idle and the
                # sync queue also carries the x loads
                deng = nc.sync if (ci_ + ct) % 2 == 0 else nc.gpsimd
                deng.dma_start(y[ct * P:(ct + 1) * P, c0:c0 + wd],
                               ysb[:, 0:wd])

            hs_prev = None     # single tile [P, CT*N], chunk k at cols k*N
            for d in range(ND):
                n = N_D[d]
                N = n * BL
                x0 = CT * OFFB[d]

                if d + 1 < ND:
                    N2 = N_D[d + 1] * BL
                    hs_next = hspool.tile([P, CT * N2], F16, tag="hs",
                                          name=f"hs_{d + 1}")
                else:
                    N2 = 0
                    hs_next = None

                def emit_stt(kk):
                    # hsum chunk kk for diag d+1: one shifted add over the
                    # padded h row (zero pads supply the boundary cells)
                    base0 = OFF2[d] - BL if d + 1 <= W - 1 else OFF2[d]
                    nc.vector.scalar_tensor_tensor(
                        out=hs_next[:, kk * N2:(kk + 1) * N2],
                        in0=hjs(kk, base0, N2),
                        scalar=0.0, op0=ALU.bypass, op1=ALU.add,
                        in1=hjs(kk, base0 + BL, N2))

                def emit_custom(pr):
                    # h chunks 2pr,2pr+1 = relu(psum + x), one DVE op
                    psv = psp[pr].rearrange("p (k q) -> p k q",
                                            k=2)[:, :, 0:N]
                    xv = xsb[:, x0 + 2 * pr * N: x0 + (2 * pr + 2) * N]
                    xv = xv.rearrange("p (k q) -> p k q", k=2)
                    nc.vector._custom_dve(relu_add,
                                          out=hj2(2 * pr, OFF2[d], N),
                                          in0=psv, in1=xv)

                if d == 0:
                    for pr in range(2):
                        xv = xsb[:, x0 + 2 * pr * N: x0 + (2 * pr + 2) * N]
                        xv = xv.rearrange("p (k q) -> p k q", k=2)
                        nc.scalar.activation(hj2(2 * pr, OFF2[d], N), xv,
                                             ACTF.Relu)
                    if hs_next is not None:
                        for kk in range(CT):
                            emit_stt(kk)
                else:
                    psp = [recps.tile([P, 2 * SLOT], F32, tag="ps",
                                      name=f"ps{d}_{pr}")
                           for pr in range(2)]

                    def ps_out(g, width):
                        return psp[g // 2][:, (g % 2) * SLOT:
                                           (g % 2) * SLOT + width]

                    # Half-split emission: all chunk-0/1 matmuls first (in
                    # k order, matching the previous diagonal's hsum-chunk
                    # production order) so the pair-A psum closes at matmul
                    # #8 — its custom-relu and the first hsum STTs then
                    # fully overlap the chunk-2/3 matmul half, driving the
                    # cross-diagonal PE gap toward zero.
                    for k in range(CT):
                        for g in (0, 1):
                            nc.tensor.matmul(
                                ps_out(g, N),
                                lhsT=w_slice(whh_sb, k, g),
                                rhs=hs_prev[:, k * N_: (k + 1) * N_],
                                start=(k == 0), stop=(k == CT - 1))
                    emit_custom(0)
                    if hs_next is not None:
                        emit_stt(0)
                        emit_stt(1)
                    for k in range(CT):
                        for g in (2, 3):
                            nc.tensor.matmul(
                                ps_out(g, N),
                                lhsT=w_slice(whh_sb, k, g),
                                rhs=hs_prev[:, k * N_: (k + 1) * N_],
                                start=(k == 0), stop=(k == CT - 1))
                    emit_custom(1)
                    if hs_next is not None:
                        emit_stt(2)
                        emit_stt(3)

                hs_prev = hs_next
                N_ = N2

                # inject the remaining pad memsets into the first
                # diagonals (one chunk per diagonal, alternating engines):
                # the wavefront only reaches pad column OFF2[d+2] many
                # diagonals later, and these early diagonals have big
                # engine slack
                if 1 <= d <= CT:
                    k = d - 1
                    # Pool has nothing per-diagonal to do — zero wavefront
                    # impact there
                    nc.gpsimd.memset(
                        hj[:, k * TOT2 + MPFX: k * TOT2 + MZERO], 0.0)

                # inline y work: up to two ready ct-groups per diagonal
                emitted = 0
                while (y_pos < len(y_queue) and emitted < 3
                       and YREADY[y_queue[y_pos][0]] + YLAG <= d):
                    emit_y_group(*y_queue[y_pos])
                    y_pos += 1
                    emitted += 1

            ti = 0
            while y_pos < len(y_queue):
                emit_y_group(*y_queue[y_pos],
                             pool="r" if ti % 2 == 1 else None, tail=True)
                y_pos += 1
                ti += 1

    nc.compile()
    return nc


_CACHE = {}


def _get_program():
    if "nc" not in _CACHE:
        _CACHE["nc"] = _build_program()
    return _CACHE["nc"]


def _host_indices():
    """Precompute gather indices for host-side pre/post permutation."""
    if "idx" in _CACHE:
        return _CACHE["idx"]
    ct_of = np.empty(CT * TOT, dtype=np.int64)
    cell_of = np.empty(CT * TOT, dtype=np.int64)
    b_of = np.empty(CT * TOT, dtype=np.int64)
    cell_base = 0
    for d in range(ND):
        n = N_D[d]
        q0 = CT * OFFB[d]
        blk = n * BL
        for ct in range(CT):
            qs = q0 + ct * blk
            idx = np.arange(blk)
            ct_of[qs:qs + blk] = ct
            cell_of[qs:qs + blk] = cell_base + idx // BL
            b_of[qs:qs + blk] = idx % BL
        cell_base += n
    ci = np.empty(H * W, dtype=np.int64)
    cj = np.empty(H * W, dtype=np.int64)
    qcell = np.empty((H, W), dtype=np.int64)
    cell_base = 0
    for d in range(ND):
        for s in range(N_D[d]):
            i = IMIN[d] + s
            ci[cell_base] = i
            cj[cell_base] = d - i
            qcell[i, d - i] = OFF2[d] + s * BL
            cell_base += 1
    _CACHE["idx"] = (ct_of, cell_of, b_of, ci, cj, qcell)
    return _CACHE["idx"]


def make_in_maps(x, whh, wyh, b):
    ct_of, cell_of, b_of, ci, cj, qcell = _host_indices()
    whh16 = whh.astype(np.float16)
    wyh16 = wyh.astype(np.float16)
    biasp = np.ascontiguousarray(b.reshape(CT, P).T.astype(np.float32))
    xg = x[:, :, ci, cj]                             # [B, C, 1024]
    in_maps = []
    for c in range(NCORES):
        arr = xg[c * BL:(c + 1) * BL]                # [BL, C, 1024]
        arr3 = arr.reshape(BL, CT, P, H * W).transpose(2, 1, 3, 0)
        xs_core = np.ascontiguousarray(
            arr3[np.arange(P)[:, None], ct_of[None, :], cell_of[None, :],
                 b_of[None, :]].astype(np.float16))
        in_maps.append({"xs": xs_core, "whh": whh16, "wyh": wyh16,
                        "biasp": biasp})
    return in_maps


def kernel(input, weight_hh, weight_yh, bias):
    x = np.ascontiguousarray(np.asarray(input, dtype=np.float32))
    whh = np.asarray(weight_hh, dtype=np.float32)
    wyh = np.asarray(weight_yh, dtype=np.float32)
    b = np.asarray(bias, dtype=np.float32)

    nc = _get_program()
    in_maps = make_in_maps(x, whh, wyh, b)
    res = bass_utils.run_bass_kernel_spmd(nc, in_maps,
                                          core_ids=list(range(NCORES)))

    _, _, _, _, _, qcell = _host_indices()
    out = np.empty((B, C, H, W), dtype=np.float32)
    qidx = qcell[None, :, :] + np.arange(BL)[:, None, None]
    for c in range(NCORES):
        ydev = res.results[c]["y"]                   # [512, 4352] f16
        out[c * BL:(c + 1) * BL] = (
            ydev[:, qidx].transpose(1, 0, 2, 3).astype(np.float32))
    return out
